# revision 32
# baseline (speedup 1.0000x reference)
"""CKConv (continuous-kernel causal conv) Trainium2 Bass kernel.

Problem: out[b,o,t] = sum_{ci,k<=t} g[o,ci,k] * x[b,ci,t-k] + bias[o]
with g generated by a tiny SIREN net on relative positions.
Shapes: B=4, CIN=32, COUT=64, T=2048, kernel length K=T+1 (tap 2048 never
contributes for t < T, so only taps 0..2047 are computed).

Sharding: 8 cores = (batch b in 0..3) x (input-channel half h in 0..1).
Each core computes a partial over its 16 input channels for all 64 output
channels; the host adds the two halves and the bias (exact fp32 adds).

Conv formulation (x-stationary): time tiles of 128. For output tile tt and
tap tile j, the contribution is Xwin(d=tt-j).T @ G(j) where Xwin(d)[r, tloc]
= xpad(128d + tloc + r - 127) is a 128x128 window of the shifted-replicated
input (im2col by a single overlapping-window DMA from the host-prepadded
bf16 input, partition step +1), and G(j)[r, o] = g[o, cl, 128j + 127 - r].
The within-tile tap reversal comes free from a block-reversed position
index fed to the SIREN.  One matmul per (cl, w, d) covers all valid beta
blocks at once (moving operand with 2 free dims).

Output accumulates in 2 PSUM banks (w=0: t in [0,1024), w=1: [1024,2048))
but w=1 drains in two halves -- B: [1024,1536) completes at (cl15,d11) and
C: [1536,2048) at the very end -- so most drain work (PSUM->SBUF cast, PE
transposes, copies, DMA out) hides under the conv tail.  Banks are memset
once and all conv matmuls accumulate (start=False): a start=True wipes the
entire bank, and the scheduler's reordering makes "first touch" fragile.

SIREN is packed across partitions to kill head latency: positions come
from an on-chip iota (block-reversed index folded into the ACT scale/
bias), h1 as [64, 512] (4 position blocks x 16 chans), h2 via a block-
diagonal [64,128] stationary into [128, 512] where each 32-partition block
holds 16 d2 rows + a ones row (ACT Sin with bias pi/2 on a zero input).
Gt2 contracts all 128 partitions against a 4x-replicated zero-padded w3 so
the padding rows vanish.  All g coefficients live in ONE [128, 16384] tile
so each Gt2 drain is a single whole-pg copy (engines alternate per jt,
~310ns/jt effective vs ~470 split).  Conv for cl0 is interleaved into the
Gt2 half-0 emission (each chunk needs only already-drained taps), keeping
the PE dense through the HAM warmup so the clock gate never re-clamps.

Matmul dtype bfloat16: ~3e-3 max-rel / ~3e-4 rms-rel error.
"""

import numpy as np

B, CIN, COUT, T = 4, 32, 64, 2048
DK = 16
N_CORES = 8
CPC = CIN // 2          # channels per core = 16
XPAD_W = 2560           # 512 left zeros + 2048 data (host pre-padded)
XC_W = 2432             # im2col window columns
NW1 = 6                 # HAM warmup matmuls before the h2 matmul
NW2 = 6                 # bridge matmuls covering the h2 Sin window


def _build_program(om2: float, dt_conv_name: str):
    import concourse.bass as bass
    import concourse.mybir as mybir
    import concourse.tile as tile
    from concourse import bacc
    from concourse.masks import make_identity

    F32 = mybir.dt.float32
    F32R = mybir.dt.float32r
    DTC = getattr(mybir.dt, dt_conv_name)
    AF = mybir.ActivationFunctionType

    nc = bacc.Bacc("TRN2", target_bir_lowering=False, debug=False,
                   num_devices=N_CORES)

    xsp = nc.dram_tensor("xsp", [CPC, XPAD_W], DTC, kind="ExternalInput")
    pf32 = nc.dram_tensor("pf32", [128, 131], F32, kind="ExternalInput")
    pbf = nc.dram_tensor("pbf", [32, 1024], DTC, kind="ExternalInput")
    y = nc.dram_tensor("y", [COUT, T], F32, kind="ExternalOutput")

    with tile.TileContext(nc) as tc:
        with tc.tile_pool(name="const", bufs=1) as const, \
             tc.tile_pool(name="sb", bufs=1) as sb, \
             tc.tile_pool(name="sbd", bufs=3) as sbd, \
             tc.tile_pool(name="outp", bufs=3) as outp, \
             tc.tile_pool(name="gt", bufs=1) as gtp, \
             tc.tile_pool(name="xcp", bufs=3) as xcp, \
             tc.tile_pool(name="psg", bufs=4, space="PSUM") as psg, \
             tc.tile_pool(name="psc", bufs=1, space="PSUM") as psc, \
             tc.tile_pool(name="pst", bufs=2, space="PSUM") as pst:

            # ---------- head: warm source + ACT Sin-table preload ----------
            warm = const.tile([128, 512], DTC, name="warm")
            nc.gpsimd.memset(warm[:].bitcast(F32), 0.0)
            sintab = const.tile([DK, 4], F32, name="sintab")
            nc.scalar.activation(sintab[:], warm[0:DK, 0:4], AF.Sin)

            # block-reversed position index: k0[tl] = 128*(tl//128)+127-tl%128
            k0f = const.tile([64, 512], F32, name="k0f")
            nc.gpsimd.iota(k0f[:], pattern=[[128, 4], [-1, 128]], base=127,
                           channel_multiplier=0,
                           allow_small_or_imprecise_dtypes=True)

            # ---------- param + first im2col DMAs (sync queue; all small,
            # all land well before their consumers) ----------
            # pw3pad rows 17:128 must read as zeros for the 128-row
            # replication contraction: memset first, DMA the 17 live rows
            pw3pad_t = const.tile([128, 1024], DTC, name="pw3pad")
            # zero only rows 32:128 (engine partition offsets must be
            # 32-aligned); rows 17:32 arrive zeroed in the DMA itself,
            # which stays disjoint so it isn't serialized behind the memset
            nc.vector.memset(pw3pad_t[32:64, :].bitcast(F32), 0.0)
            nc.vector.memset(pw3pad_t[64:96, :].bitcast(F32), 0.0)
            nc.vector.memset(pw3pad_t[96:128, :].bitcast(F32), 0.0)
            pf32_t = const.tile([128, 131], F32)
            nc.sync.dma_start(out=pf32_t[:], in_=pf32.ap())
            nc.sync.dma_start(out=pw3pad_t[0:32, :], in_=pbf.ap())

            xcts = {}

            def ensure_xc(cl):
                if cl in xcts or cl >= CPC:
                    return
                t = xcp.tile([128, XC_W], DTC)
                nc.sync.dma_start(
                    out=t[:],
                    in_=bass.AP(xsp, cl * XPAD_W + 1, [[1, 128], [1, XC_W]]))
                xcts[cl] = t

            ensure_xc(0)
            ensure_xc(1)
            ensure_xc(2)

            b2v2 = pf32_t[:, 0:1]
            h1sc = pf32_t[0:64, 1:2]
            h1bi = pf32_t[0:64, 2:3]
            # W2big ships inside pf32 (f32) and is cast to the conv dtype
            w2big = sb.tile([64, 128], DTC, name="w2big")
            nc.vector.tensor_copy(w2big[:], pf32_t[0:64, 3:131])
            pw3pad = pw3pad_t[:]

            # transpose identity, also used (in DTC) to replicate w3aug
            identf = const.tile([128, 128], F32, name="identf")
            make_identity(nc, identf[:])
            identb = const.tile([128, 128], DTC, name="identb")
            nc.vector.tensor_copy(identb[:], identf[:])
            # tb=3 variant: identity block at cols 96..113 (out partition
            # base is restricted to 0/32/64, so shift columns instead)
            identb3 = const.tile([128, 114], DTC, name="identb3")
            nc.vector.memset(identb3[:].bitcast(F32), 0.0)
            nc.vector.tensor_copy(identb3[:, 96:113], identf[:, 0:17])
            ident = const.tile([128, 128], F32R, name="ident")
            nc.vector.tensor_copy(ident[:], identf[:])

            # w3sel [128, 4096]: 4 tb blocks x (2 halves x 512); zeros
            # outside the 17 live rows per 32-block
            w3sel_t = sb.tile([128, 4096], DTC, name="w3sel")
            nc.vector.memset(w3sel_t[:].bitcast(F32), 0.0)
            repl_done = 0

            def emit_repl(k):
                # one (tb, half) block: psum[32tb+d, c] = pw3[d, half*512+c]
                # via identity stationary (full-128 contraction keeps the
                # HAM activity monitor fed with real work)
                tb, half = k % 4, k // 4
                ps = psg.tile([128, 512], F32, tag="g")
                if tb < 3:
                    nc.tensor.matmul(ps[32 * tb:32 * tb + 17, :],
                                     identb[:, 0:17],
                                     pw3pad[:, half * 512:(half + 1) * 512],
                                     start=True, stop=True)
                else:
                    nc.tensor.matmul(ps[0:113, :], identb3[:, 0:113],
                                     pw3pad[:, half * 512:(half + 1) * 512],
                                     start=True, stop=True)
                src_ = ps[32 * tb:32 * tb + 17, :]
                dst = w3sel_t[32 * tb:32 * tb + 17,
                              1024 * tb + 512 * half:
                              1024 * tb + 512 * half + 512]
                if k % 2 == 0:
                    nc.vector.tensor_copy(dst, src_)
                else:
                    nc.scalar.copy(dst, src_)

            # ---------- conv accumulators: memset + accumulate-only ----------
            pA = psc.tile([128, 512], F32, name="pA")
            pBC = psc.tile([128, 512], F32, name="pBC")
            nc.vector.memset(pA[:], 0.0)
            nc.vector.memset(pBC[:], 0.0)

            # ---------- HAM warmup burst (cold ~427ns each) ----------
            pwarm = psg.tile([128, 512], F32, tag="g")
            for i in range(NW1):
                nc.tensor.matmul(pwarm[:], warm[:, 0:128], warm[:],
                                 start=(i == 0), stop=(i == NW1 - 1),
                                 skip_group_check=True)

            last_pg = [None]

            def emit_filler(n=1, cols=256):
                # pure PE activity to keep the HAM window busy: either a
                # zero-accumulating matmul into the live pg (data unchanged,
                # warm is all-zero) or bare weight loads (no PSUM touched)
                for _ in range(n):
                    if last_pg[0] is None:
                        nc.tensor.ldweights(warm[:, 0:128])
                        nc.tensor.ldweights(warm[:, 0:128])
                    else:
                        nc.tensor.matmul(last_pg[0][:, 0:cols],
                                         warm[:, 0:128], warm[:, 0:cols],
                                         start=False, stop=False,
                                         skip_group_check=True)

            # ---------- SIREN, partition-packed ----------
            # h1[(tb,d1), tl] = sin(om1*(w1[d1]*p + b1[d1])),
            # p = (tb/2 - 1) + k0/1024 folded into per-partition scale/bias
            h1b = sb.tile([64, 512], DTC)
            nc.scalar.activation(h1b[:], k0f[:], AF.Sin,
                                 bias=h1bi, scale=h1sc)
            # h2p[(tb,d2'), tl] = sum_d1 w2[d2',d1] h1[(tb,d1), tl]
            # (block-diagonal stationary; d2'=16 ones-row and pad rows get 0)
            # -- emitted before the repls, which depend on the slower pw3 DMA
            # (bare weight loads pad the variable h1-latency window first)
            for _ in range(14):
                nc.tensor.ldweights(warm[:, 0:128])
            h2p = psg.tile([128, 512], F32, tag="g")
            nc.tensor.matmul(h2p[:], w2big[:], h1b[:], start=True, stop=True)
            # replicate w3aug's half-0 blocks (doubles as HAM warm work and
            # covers the h2 Sin window on the PE)
            for k in range(5):
                emit_repl(k)
                emit_filler(1)
            h2r = sb.tile([128, 512], DTC)
            nc.scalar.activation(h2r[:], h2p[:], AF.Sin,
                                 bias=b2v2, scale=float(om2))

            # ---------- Gt2 into one tile: gtall[r, q, j, (cl%4)*64+o] ----------
            gtall = gtp.tile([128, 4 * 16 * 256], DTC, name="gtall")
            gtv = gtall[:].rearrange("p (q j x) -> p q j x", q=4, j=16)

            drain_ctr = [0]

            def emit_gt2(half, jts, fillers=False):
                for jt in jts:
                    pg = psg.tile([128, 512], F32, tag="g")
                    nc.tensor.matmul(
                        pg[:],
                        h2r[:, (jt % 4) * 128:(jt % 4) * 128 + 128],
                        w3sel_t[:, (jt // 4) * 1024 + half * 512:
                                (jt // 4) * 1024 + half * 512 + 512],
                        start=True, stop=True)
                    last_pg[0] = pg
                    if fillers:
                        emit_filler(1)
                    # one whole-pg drain per jt (both quartets via a 2-dim
                    # dest AP); engines alternate so the per-op overhead of
                    # the PSUM-source 1x mode is paid once per 512 cols
                    src = pg[:].rearrange("p (two x) -> p two x", two=2)
                    dst = gtv[:, 2 * half:2 * half + 2, jt, :]
                    drain_ctr[0] += 1
                    if drain_ctr[0] % 2 == 0:
                        nc.vector.tensor_copy(dst, src)
                    else:
                        nc.scalar.copy(dst, src)

            def emit_conv(cl, grp, dlist=None):
                xc = xcts[cl]
                q, clq = divmod(cl, 4)
                if grp == 'A':          # w=0: tt = beta, t in [0, 1024)
                    for d in (dlist if dlist is not None else range(8)):
                        beta0 = d
                        nb = 8 - beta0
                        station = xc[:, 128 * d + 384: 128 * d + 512]
                        moving = gtv[:, q, 0:nb, clq * 64:(clq + 1) * 64]
                        nc.tensor.matmul(
                            pA[:, beta0 * 64: 512], station, moving,
                            start=False,
                            stop=(cl == CPC - 1 and d == 7),
                            skip_group_check=True)
                else:                   # w=1: tt = 8+beta, t in [1024, 2048)
                    for d in (dlist if dlist is not None else range(16)):
                        beta0 = max(0, d - 8)
                        nb = 8 - beta0
                        j0 = 8 + beta0 - d
                        station = xc[:, 128 * d + 384: 128 * d + 512]
                        moving = gtv[:, q, j0:j0 + nb,
                                     clq * 64:(clq + 1) * 64]
                        nc.tensor.matmul(
                            pBC[:, beta0 * 64: 512], station, moving,
                            start=False,
                            stop=(cl == CPC - 1 and d == 15),
                            skip_group_check=True)

            # ---------- Gt2 half 0 with conv cl0 interleaved: each conv
            # chunk only needs taps whose drains are already in flight, so
            # the PE stays dense while drains rate-limit the Gt2 stream ----
            emit_gt2(0, range(0, 4), fillers=True)
            emit_repl(5)
            emit_gt2(0, range(4, 8), fillers=True)
            emit_filler(2)
            emit_conv(0, 'W1', dlist=range(15, 7, -1))   # j <= 7
            emit_repl(6)
            emit_gt2(0, range(8, 12), fillers=True)
            emit_filler(2)
            emit_conv(0, 'A')                            # j <= 7
            emit_repl(7)
            emit_gt2(0, range(12, 16), fillers=True)
            emit_filler(2)
            emit_conv(0, 'W1', dlist=range(7, -1, -1))   # j up to 15

            for cl in range(1, 4):
                ensure_xc(cl + 2)
                emit_conv(cl, 'A')
                emit_conv(cl, 'W1')
            for cl in range(4, 8):
                ensure_xc(cl + 2)
                j0 = (cl - 4) * 4
                emit_gt2(1, range(j0, j0 + 2))
                emit_conv(cl, 'A')
                emit_gt2(1, range(j0 + 2, j0 + 4))
                emit_conv(cl, 'W1')
            for cl in range(8, CPC - 2):
                ensure_xc(cl + 2)
                emit_conv(cl, 'A')
                emit_conv(cl, 'W1')

            # ---------- cl14/cl15: drains spread through the w1 stream ----
            cl14, cl15 = CPC - 2, CPC - 1
            emit_conv(cl14, 'A')
            emit_conv(cl15, 'A')            # pA complete
            emit_conv(cl14, 'W1', dlist=range(0, 8))

            sb_dA = sbd.tile([128, 512], F32R, name="sbdA")
            nc.vector.tensor_copy(sb_dA[:], pA[:])
            outA = outp.tile([COUT, 1024], F32, name="outA")

            def tr_block(src_sb, out_sb, b8, eng):
                pt = pst.tile([COUT, 128], F32R)
                nc.tensor.transpose(pt[:], src_sb[:, b8 * 64:(b8 + 1) * 64],
                                    ident[:])
                dst = out_sb[:, b8 * 128:(b8 + 1) * 128]
                if eng == 0:
                    nc.vector.tensor_copy(dst, pt[:])
                else:
                    nc.scalar.copy(dst, pt[:])

            emit_conv(cl14, 'W1', dlist=range(8, 16))
            for b8 in range(4):
                tr_block(sb_dA, outA, b8, b8 % 2)
            emit_conv(cl15, 'W1', dlist=range(0, 4))
            for b8 in range(4, 8):
                tr_block(sb_dA, outA, b8, b8 % 2)
            emit_conv(cl15, 'W1', dlist=range(4, 8))
            nc.sync.dma_start(out=y.ap()[:, 0:1024], in_=outA[:])

            emit_conv(cl15, 'W1', dlist=range(8, 12))   # B region complete
            sb_dB = sbd.tile([128, 256], F32R, name="sbdB")
            nc.vector.tensor_copy(sb_dB[:], pBC[:, 0:256])
            outB = outp.tile([COUT, 512], F32, name="outB")
            emit_conv(cl15, 'W1', dlist=range(12, 14))  # C1 (beta 4,5) done
            for b4 in range(4):
                tr_block(sb_dB, outB, b4, b4 % 2)
            sb_dC1 = sbd.tile([128, 128], F32R, name="sbdC1")
            nc.vector.tensor_copy(sb_dC1[:], pBC[:, 256:384])
            emit_conv(cl15, 'W1', dlist=range(14, 16))  # C2 (beta 6,7) done
            nc.sync.dma_start(out=y.ap()[:, 1024:1536], in_=outB[:])

            # C drain in halves: only C2's [128,128] chain is fully exposed
            outC = outp.tile([COUT, 512], F32, name="outC")

            def tr_block2(src_sb, sb_b, out_sb, out_b, eng):
                pt = pst.tile([COUT, 128], F32R)
                nc.tensor.transpose(pt[:],
                                    src_sb[:, sb_b * 64:(sb_b + 1) * 64],
                                    ident[:])
                dst = out_sb[:, out_b * 128:(out_b + 1) * 128]
                if eng == 0:
                    nc.vector.tensor_copy(dst, pt[:])
                else:
                    nc.scalar.copy(dst, pt[:])

            for b4 in range(2):
                tr_block2(sb_dC1, b4, outC, b4, b4 % 2)
            sb_dC2 = sbd.tile([128, 128], F32R, name="sbdC2")
            nc.vector.tensor_copy(sb_dC2[:], pBC[:, 384:512])
            for b4 in range(2):
                tr_block2(sb_dC2, b4, outC, 2 + b4, b4 % 2)
            nc.sync.dma_start(out=y.ap()[:, 1536:2048], in_=outC[:])

    nc.compile()
    return nc


def kernel(x, pos_rel, w1, b1, om1, w2, b2, om2, w3, b3, bias,
           dt_conv_name: str = "bfloat16", _trace_tmpdir=None):
    import ml_dtypes
    from concourse.bass_utils import run_bass_kernel_spmd

    x = np.asarray(x, dtype=np.float32)
    pos_rel = np.asarray(pos_rel, dtype=np.float32)
    w1 = np.asarray(w1, dtype=np.float32)
    b1 = np.asarray(b1, dtype=np.float32)
    om1 = float(np.asarray(om1))
    w2 = np.asarray(w2, dtype=np.float32)
    b2 = np.asarray(b2, dtype=np.float32)
    om2 = float(np.asarray(om2))
    w3 = np.asarray(w3, dtype=np.float32)
    b3 = np.asarray(b3, dtype=np.float32)
    bias = np.asarray(bias, dtype=np.float32)
    bf16 = ml_dtypes.bfloat16

    # pf32 [128, 3]: col0 = b2 bias per (tb,d2') 32-block (pi/2 on ones/pad
    # rows); col1/col2 = h1 ACT scale/bias with the iota position index
    # folded in: p = (tb/2 - 1) + k0/1024
    w1f = w1.reshape(DK)
    pf32 = np.zeros((128, 131), np.float32)
    pf32[:, 0] = np.pi / 2
    for tb in range(4):
        pf32[32 * tb:32 * tb + 16, 0] = om2 * b2
        pf32[16 * tb:16 * tb + 16, 1] = om1 * w1f / 1024.0
        pf32[16 * tb:16 * tb + 16, 2] = om1 * (w1f * (tb / 2.0 - 1.0) + b1)

    # W2big [64, 128]: block-diagonal w2.T; cols (tb,16..31) zero;
    # shipped in pf32 cols 3:131
    w2big = np.zeros((64, 128), np.float32)
    for tb in range(4):
        w2big[16 * tb:16 * tb + 16, 32 * tb:32 * tb + 16] = w2.T
    pf32[0:64, 3:131] = w2big

    nc = _build_program(om2, dt_conv_name)

    in_maps = []
    for core in range(N_CORES):
        b, h = divmod(core, 2)
        ci0 = h * CPC
        # w3a[d, cl*64 + o] = w3[o*CIN + ci0 + cl, d]; b3a = matching b3 row
        w3_r = w3.reshape(COUT, CIN, DK)[:, ci0:ci0 + CPC, :]
        w3a = np.transpose(w3_r, (2, 1, 0)).reshape(DK, CPC * COUT)
        b3_r = b3.reshape(COUT, CIN)[:, ci0:ci0 + CPC]
        b3a = np.transpose(b3_r, (1, 0)).reshape(CPC * COUT)

        # pbf [32, 1024]: compact w3aug (16 w3 rows + the b3 row + zeros)
        pbf = np.zeros((32, 1024), np.float32)
        pbf[0:16, :] = w3a
        pbf[16, :] = b3a

        xsp = np.zeros((CPC, XPAD_W), np.float32)
        xsp[:, 512:] = x[b, ci0:ci0 + CPC, :]

        in_maps.append({
            "xsp": xsp.astype(bf16),
            "pf32": pf32,
            "pbf": pbf.astype(bf16),
        })

    kwargs = {}
    if _trace_tmpdir is not None:
        kwargs = dict(trace=True, tmpdir=_trace_tmpdir)
    res = run_bass_kernel_spmd(nc, in_maps, list(range(N_CORES)), **kwargs)

    out = np.empty((B, COUT, T), dtype=np.float32)
    for b in range(B):
        out[b] = res.results[2 * b]["y"] + res.results[2 * b + 1]["y"]
    out += bias[None, :, None]
    if _trace_tmpdir is not None:
        kernel.last_exec_time_ns = res.exec_time_ns
    return out
